# revision 38
# baseline (speedup 1.0000x reference)
"""MinGRU layer Trainium2 kernel (v3: dense down-sweep, blocked ev/od output).

Reference semantics (B=8, T=16384, D=H=O=256):
    zs = sigmoid(xs @ Wz.T + bz);  hs = xs @ Wh.T + bh
    a = concat([1], 1-zs);  b = concat([0], zs*hs)         (T+1 positions)
    states = jax.lax.associative_scan(combine, (a, b))[1][:, 1:]
    out = states @ Wo.T + bo
with combine((a0,b0),(a1,b1)) = (a0*b0, b0*a1 + b1) — NOT associative; the
result is defined by jax's odd/even recursion tree, replicated exactly
(modulo dropping |A|-products at tree level >= 6, which are ~3e-4).

v3 layout/scheduling notes (vs v2):
  - Packed halves [128, pos, 2] bf16 everywhere (2x_1p DVE mode).  Probe
    measurements: stride-2-position access keeps full DVE speed (TT 2x,
    TS/copy 4x); stride >= 4-position drops to 1x.  The v2 down-sweep wrote
    dbuf at strides 4..4096 (1x); v3 keeps every access dense or stride-2.
  - Down-sweep: dense cascade R_6..R_1 where R_l[i] = scan value at chunk
    position i*2^l - 1 (R_l[0] = chunk prefix).  R_6 = [prefix, B_6];
    level l: odds of R_l by FMA from R_{l+1} (stride-2 writes, 2x), evens
    by copy of R_{l+1} (4x).  Final level-0 FMA produces dbuf_f (dense).
  - Output is stored BLOCKED per 512-col sub: first 256 psum cols = even
    scan positions (from dbuf_f), next 256 = odd (from dbuf_r = R_1); the
    host de-interleaves (free) and adds bo.
  - a = 1-z on DVE tensor_scalar (4x) instead of ACT sigmoid(-x) (frees ACT).
  - b = z*h: pipeline-head subs (chunk 0 s<3) via DVE STT straight from
    PSUM (short critical chain); everywhere else ACT evacuates yh+bh and
    DVE does a 2x TT (2.3x cheaper per sub on DVE).
  - psum_y/psum_o both double-buffered (4+4 of the 8 PSUM banks) so the
    out-matmuls of sub s+1 overlap sub s's evacuation.
  - Chunk c's out stage (matmuls+evac+DMA) is emitted one loop iteration
    LATE, after phase1(c+1): its ops are then data-ready when the in-order
    PE/ACT queues reach them, so an out burst waiting on the down-sweep
    never head-blocks the next chunk's phase-1 stream.
  - Head DMAs issue from the ACT/GPSIMD queues (parallel issue, the SP
    queue serializes at ~640ns per DMA).
Measured (8 cores, NTFF): ~180-181us vs 190.2us for v2, rel err 0.00570
(identical to v2 -- the restructure is numerically exact).  Beware: pool
depth changes (zy 4->5, x 4->6) regress 10-30us via SBUF layout shifts,
and the machine occasionally has ~212us slow episodes unrelated to code.
Sharding: batch b=8 across the 8 cores (one sequence per core); weights
replicated.  The host pre-transposes/casts x and the weights.
"""

from contextlib import ExitStack

import numpy as np
import ml_dtypes

import concourse.bacc as bacc
import concourse.tile as tile
from concourse import mybir
from concourse.bass_utils import run_bass_kernel_spmd

BF16 = ml_dtypes.bfloat16
F32 = mybir.dt.float32
BF = mybir.dt.bfloat16

B, T, D, H, O = 8, 16384, 256, 256, 256
L = 4096          # positions per chunk (power of 2)
NCHUNK = T // L   # 4 full chunks; position T (=16384) handled separately
SUB = 512         # matmul sub-chunk (one PSUM bank at f32)
NSUB = L // SUB   # 8
ADROP = 5         # tree levels >= ADROP: |A| products are dropped
                  # (bf16-sim rel err 0.0088 vs 0.0041 at ADROP=6; gate 2e-2)

AluOp = mybir.AluOpType
ActFn = mybir.ActivationFunctionType

# level arrays for lvl 1..ADROP (Aup/Bup): sizes L/2..L/64
LVL_OFF = {}
_o = 0
for _l in range(1, ADROP + 1):
    LVL_OFF[_l] = _o
    _o += L >> _l
LVL_TOTAL = _o  # 4032


def build_nc():
    nc = bacc.Bacc()

    xt = nc.dram_tensor("xt", [D, T], BF, kind="ExternalInput")
    wall = nc.dram_tensor("wall", [D, 3 * H], BF, kind="ExternalInput")
    ball = nc.dram_tensor("ball", [H, 3], F32, kind="ExternalInput")
    out = nc.dram_tensor("out", [O, T], BF, kind="ExternalOutput")

    with tile.TileContext(nc) as tc, ExitStack() as ctx:
        singles = ctx.enter_context(tc.tile_pool(name="singles", bufs=1))
        ab_pool = ctx.enter_context(tc.tile_pool(name="ab", bufs=2))
        lvl_pool = ctx.enter_context(tc.tile_pool(name="lvl", bufs=1))
        dbuf_pool = ctx.enter_context(tc.tile_pool(name="dbuf", bufs=2))
        rsm_pool = ctx.enter_context(tc.tile_pool(name="rsm", bufs=2))
        tmp_pool = ctx.enter_context(tc.tile_pool(name="tmp", bufs=2))
        zy_pool = ctx.enter_context(tc.tile_pool(name="zy", bufs=4))
        x_pool = ctx.enter_context(tc.tile_pool(name="xp", bufs=4))
        osb_pool = ctx.enter_context(tc.tile_pool(name="osb", bufs=3))
        psum_y = ctx.enter_context(tc.tile_pool(name="psy", bufs=2, space="PSUM"))
        psum_o = ctx.enter_context(tc.tile_pool(name="pso", bufs=2, space="PSUM"))

        # ---- prefetch the first x sub-tile before the weight DMAs so its
        # transfer overlaps them (shortens the head-of-kernel latency) ----
        # head DMAs issue from different engine queues so they don't
        # serialize behind each other on the SP queue (~640ns issue each)
        xk_pre = []
        for s0, eng in ((0, nc.gpsimd), (1, nc.scalar)):
            xp = x_pool.tile([128, 2, SUB], BF, name="xk", tag="xk")
            eng.dma_start(
                out=xp[:, :, :SUB],
                in_=xt[:, s0 * SUB:(s0 + 1) * SUB].rearrange(
                    "(k p) n -> p k n", p=128))
            xk_pre.append(xp)

        # ---- constants: one DMA for all weights, one for all biases ----
        wsb = singles.tile([128, 2, 3 * H], BF, name="wsb")
        nc.scalar.dma_start(out=wsb,
                            in_=wall[:, :].rearrange("(k p) n -> p k n", p=128))
        wz_sb = [wsb[:, k, 0:H] for k in range(2)]
        wh_sb = [wsb[:, k, H:2 * H] for k in range(2)]
        wo_sb = [wsb[:, k, 2 * H:3 * H] for k in range(2)]
        bsb = singles.tile([128, 2, 3], F32, name="bsb")
        nc.sync.dma_start(out=bsb,
                          in_=ball[:, :].rearrange("(h p) n -> p h n", p=128))
        bz_sb = [bsb[:, h, 0:1] for h in range(2)]
        bzn_sb = [bsb[:, h, 1:2] for h in range(2)]
        bh_sb = [bsb[:, h, 2:3] for h in range(2)]

        # cross-chunk bookkeeping, halves packed innermost: [128, n, 2]
        otb = singles.tile([128, 4, 2], BF, name="otb")

        abufs = {}

        def emit_phase1(c, s_lo=0, s_hi=NSUB):
            """DMA x, matmuls, sigmoid z, a = 1-z (DVE TS), b = z*h for
            chunk c, subs [s_lo, s_hi).  Chunk 0 runs b on DVE (STT from
            PSUM, per sub) to shorten the head critical chain; chunks >=1
            use ACT yh-evac + DVE TT except the last sub (STT)."""
            if s_lo == 0:
                a_buf = ab_pool.tile([128, L, 2], BF, name="a_buf", tag="a")
                b_buf = ab_pool.tile([128, L, 2], BF, name="b_buf", tag="b")
                abufs[c] = (a_buf, b_buf)
                if c == 0:
                    nc.vector.memset(a_buf[:, 0:1, :], 1.0)
                    nc.vector.memset(b_buf[:, 0:1, :], 0.0)
            else:
                a_buf, b_buf = abufs[c]

            for s in range(s_lo, s_hi):
                if c == 0:
                    x0 = s * SUB
                    ncols = SUB if s < NSUB - 1 else SUB - 1
                    acol = s * SUB + 1
                else:
                    x0 = c * L - 1 + s * SUB
                    ncols = SUB
                    acol = s * SUB
                if c == 0 and s < 2:
                    xk = xk_pre[s]
                else:
                    xk = x_pool.tile([128, 2, SUB], BF, name="xk", tag="xk")
                    nc.sync.dma_start(
                        out=xk[:, :, :ncols],
                        in_=xt[:, x0:x0 + ncols].rearrange("(k p) n -> p k n",
                                                           p=128))
                # A,C,B,D,A,C,B,D over 4 banks: every same-bank revisit is
                # 4 apart, so no matmul pays the PSUM-bank half-rate bubble
                yzp = [psum_y.tile([128, SUB], F32, name=f"yz{h}", tag=f"y{h}")
                       for h in range(2)]
                yhp = [psum_y.tile([128, SUB], F32, name=f"yh{h}", tag=f"y{h}")
                       for h in range(2)]
                for k in range(2):
                    st, sp = k == 0, k == 1
                    for h in range(2):
                        nc.tensor.matmul(yzp[h][:, :ncols],
                                         wz_sb[k][:, h * 128:(h + 1) * 128],
                                         xk[:, k, :ncols], start=st, stop=sp)
                    for h in range(2):
                        nc.tensor.matmul(yhp[h][:, :ncols],
                                         wh_sb[k][:, h * 128:(h + 1) * 128],
                                         xk[:, k, :ncols], start=st, stop=sp)
                zt = zy_pool.tile([128, SUB, 2], BF, name="zt", tag="zt")
                # STT (1x from PSUM, no ACT dependency) only where the
                # pipeline head latency matters; TT elsewhere is 2.3x
                # cheaper on DVE
                use_stt = c == 0 and s < 3
                yhs = None
                if not use_stt:
                    yhs = zy_pool.tile([128, SUB, 2], BF, name="yhs",
                                       tag="yhs")
                for h in range(2):
                    nc.scalar.activation(zt[:, :ncols, h], yzp[h][:, :ncols],
                                         ActFn.Sigmoid, bias=bz_sb[h][:, 0:1],
                                         scale=1.0)
                    if use_stt:
                        # b = (yh+bh)*z on DVE straight from PSUM (1x)
                        nc.vector.scalar_tensor_tensor(
                            b_buf[:, acol:acol + ncols, h],
                            yhp[h][:, :ncols], bh_sb[h][:, 0:1],
                            zt[:, :ncols, h],
                            op0=AluOp.add, op1=AluOp.mult)
                    else:
                        nc.scalar.activation(yhs[:, :ncols, h],
                                             yhp[h][:, :ncols], ActFn.Identity,
                                             bias=bh_sb[h][:, 0:1], scale=1.0)
                # a = 1 - z on DVE tensor_scalar (4x mode)
                nc.vector.tensor_scalar(a_buf[:, acol:acol + ncols, :],
                                        zt[:, :ncols, :], -1.0, 1.0,
                                        op0=AluOp.mult, op1=AluOp.add)
                if not use_stt:
                    nc.vector.tensor_tensor(b_buf[:, acol:acol + ncols, :],
                                            yhs[:, :ncols, :],
                                            zt[:, :ncols, :], op=AluOp.mult)

        NSPLIT = 5  # chunk-0 up levels emitted per half (head pipelining)

        def emit_up_levels(c, Aup, Bup, levels, part=None):
            a_buf, b_buf = abufs[c]
            for lvl in levels:
                n = L >> lvl
                m = n // 2
                if lvl == 0:
                    sA, sB = a_buf, b_buf
                else:
                    o = LVL_OFF[lvl]
                    sA = Aup[:, o:o + n, :]
                    sB = Bup[:, o:o + n, :]
                o2 = LVL_OFF[lvl + 1]
                if part is None:
                    lo, hi = 0, m
                else:
                    lo, hi = (0, m // 2) if part == 0 else (m // 2, m)
                dB = Bup[:, o2 + lo:o2 + hi, :]
                A_ev = sA[:, 2 * lo:2 * hi:2, :]
                A_od = sA[:, 2 * lo + 1:2 * hi:2, :]
                B_ev = sB[:, 2 * lo:2 * hi:2, :]
                B_od = sB[:, 2 * lo + 1:2 * hi:2, :]
                if lvl + 1 <= ADROP - 1:
                    # A of the next level is consumed (up B-chain + down FMA)
                    dA = Aup[:, o2 + lo:o2 + hi, :]
                    nc.vector.tensor_tensor(dA, A_ev, B_ev, op=AluOp.mult)
                tu = tmp_pool.tile([128, L // 2, 2], BF, name="tu", tag="tmp")
                nc.vector.tensor_tensor(tu[:, lo:hi, :], B_ev, A_od,
                                        op=AluOp.mult)
                nc.vector.tensor_tensor(dB, tu[:, lo:hi, :], B_od,
                                        op=AluOp.add)

        def emit_up(c, Aup=None, Bup=None, tail_only=False):
            if Aup is None:
                Aup = lvl_pool.tile([128, LVL_TOTAL, 2], BF, name="Aup",
                                    tag="Au")
                Bup = lvl_pool.tile([128, LVL_TOTAL, 2], BF, name="Bup",
                                    tag="Bu")
            if tail_only:
                emit_up_levels(c, Aup, Bup, range(NSPLIT), part=1)
                emit_up_levels(c, Aup, Bup, range(NSPLIT, ADROP))
            else:
                emit_up_levels(c, Aup, Bup, range(ADROP))
            return Aup, Bup

        def emit_spine(c, Bup):
            # |A_12| ~ |ab|*prod(B)^11 is far below bf16 resolution, so the
            # cross-chunk combines collapse: otb[c] = B_12 = B_6[last]
            o6 = LVL_OFF[ADROP]
            n6 = L >> ADROP
            nc.vector.tensor_copy(otb[:, c:c + 1, :],
                                  Bup[:, o6 + n6 - 1:o6 + n6, :])

        def emit_down(c, Aup, Bup):
            """Dense cascade: R_l[i] = scan at chunk position i*2^l - 1
            (R_l[0] = prefix).  Returns (dbuf_f, dbuf_r, emit_f):
            dbuf_f[i] = scan[2i]  (chunk-local, i=0..L/2-1)
            dbuf_r[i] = scan[2i-1] (i=0..L/2; [0]=prefix, [L/2]=chunk top).
            All writes dense or stride-2 (full DVE rate).  The final level-0
            FMA into dbuf_f is emitted by calling emit_f(lo, hi) so the
            tail can interleave FMA slices with out-matmul subs."""
            a_buf, b_buf = abufs[c]
            n6 = L >> ADROP
            o6 = LVL_OFF[ADROP]
            r_prev = rsm_pool.tile([128, n6 + 1, 2], BF, name="r6", tag="r6")
            if c == 0:
                nc.vector.memset(r_prev[:, 0:1, :], 0.0)
            else:
                nc.vector.tensor_copy(r_prev[:, 0:1, :], otb[:, c - 1:c, :])
            nc.vector.tensor_copy(r_prev[:, 1:n6 + 1, :],
                                  Bup[:, o6:o6 + n6, :])
            for lvl in range(ADROP - 1, 0, -1):
                n = L >> lvl          # positions at this level
                m = n // 2
                o = LVL_OFF[lvl]
                A_ev = Aup[:, o:o + n:2, :] if lvl > 0 else None
                B_ev = Bup[:, o:o + n:2, :]
                rl = rsm_pool.tile([128, n + 1, 2], BF, name=f"r{lvl}",
                                   tag=f"r{lvl}") if lvl > 1 else \
                    dbuf_pool.tile([128, L // 2 + 1, 2], BF, name="dbuf_r",
                                   tag="dr")
                # odds: R_l[2i+1] = R_{l+1}[i] * A_l[2i] + B_l[2i]
                nc.vector.tensor_tensor(rl[:, 1:n + 1:2, :],
                                        r_prev[:, 0:m, :], A_ev,
                                        op=AluOp.mult)
                nc.vector.tensor_tensor(rl[:, 1:n + 1:2, :],
                                        rl[:, 1:n + 1:2, :], B_ev,
                                        op=AluOp.add)
                # evens: R_l[2i] = R_{l+1}[i]
                nc.vector.tensor_copy(rl[:, 0:n + 1:2, :], r_prev)
                r_prev = rl
            dbuf_r = r_prev
            # level 0: dbuf_f[i] = R_1[i] * a_buf[2i] + b_buf[2i] (the gate
            # arrays are filled with the chunk offset baked in, so the even
            # slots are correct for every chunk; chunk 0's scan[0]=0 falls
            # out of the a_buf[0]=1 / b_buf[0]=0 memsets).
            dbuf_f = dbuf_pool.tile([128, L // 2, 2], BF, name="dbuf_f",
                                    tag="df")

            def emit_f(lo, hi):
                ga = a_buf[:, 2 * lo:2 * hi:2, :]
                gb = b_buf[:, 2 * lo:2 * hi:2, :]
                nc.vector.tensor_tensor(dbuf_f[:, lo:hi, :],
                                        dbuf_r[:, lo:hi, :], ga,
                                        op=AluOp.mult)
                nc.vector.tensor_tensor(dbuf_f[:, lo:hi, :],
                                        dbuf_f[:, lo:hi, :], gb,
                                        op=AluOp.add)

            return dbuf_f, dbuf_r, emit_f

        def emit_out(c, dbuf_f, dbuf_r, s_lo=0, s_hi=NSUB):
            """Out-matmuls per 512-col sub in BLOCKED order: psum cols
            0..255 = even scan positions (dbuf_f), 256..511 = odd (dbuf_r).
            DRAM block for (c, s) starts at col c*L - 1 + 512*s (+1 skip
            for c0s0)."""
            last = c == NCHUNK - 1
            Q = SUB // 2  # 256
            for s in range(s_lo, s_hi):
                f_sl = dbuf_f[:, Q * s:Q * s + Q, :]
                r_sl = dbuf_r[:, Q * s + 1:Q * s + 1 + Q, :]
                if last and s % 2 == 1:
                    # tail: phase1 is over, reuse the idle psum_y banks so
                    # consecutive subs' matmuls/evacs fully overlap
                    p0 = psum_y.tile([128, SUB], F32, name="poa0", tag="y0")
                    p1 = psum_y.tile([128, SUB], F32, name="poa1", tag="y1")
                    po_h = [p0, p1]
                else:
                    po = psum_o.tile([128, 2, SUB], F32, name="po", tag="po")
                    po_h = [po[:, 0, :], po[:, 1, :]]
                # complete each psum column-region's k0->k1 accumulation
                # before starting the other region of the same bank (an
                # interleaved second `start` clobbers the pending group)
                for sl, q0 in ((f_sl, 0), (r_sl, Q)):
                    for k in range(2):
                        st, sp = k == 0, k == 1
                        for oh in range(2):
                            nc.tensor.matmul(
                                po_h[oh][:, q0:q0 + Q],
                                wo_sb[k][:, oh * 128:(oh + 1) * 128],
                                sl[:, :, k], start=st, stop=sp)
                base = c * L - 1 + s * SUB
                skip = 1 if (c == 0 and s == 0) else 0
                dst = out[:, base + skip:base + SUB]
                if last:
                    # tail: split every sub's evacuation across ACT and the
                    # now-idle DVE into one tile (single DMA per sub), and
                    # alternate the DMA issue queue (SP serializes issues at
                    # ~640ns each)
                    o2 = osb_pool.tile([128, 2, SUB], BF, name="osb2",
                                       tag="osb")
                    nc.scalar.copy(o2[:, 0, :], po_h[0])
                    nc.vector.tensor_copy(o2[:, 1, :], po_h[1])
                    nc.sync.dma_start(
                        out=dst.rearrange("(two p) n -> p two n", p=128),
                        in_=o2[:, :, skip:])
                else:
                    osb = osb_pool.tile([128, 2, SUB], BF, name="osb",
                                        tag="osb")
                    nc.scalar.copy(osb, po)
                    nc.sync.dma_start(
                        out=dst.rearrange("(two p) n -> p two n", p=128),
                        in_=osb[:, :, skip:])

        def emit_final():
            # final position T: scan[T] = otb[3]*a_T + b_T fed to Wo;
            # out col T-1 written directly (host treats it as a singleton).
            xl = singles.tile([128, 2, 1], BF, name="xl")
            nc.sync.dma_start(
                out=xl,
                in_=xt[:, T - 1:T].rearrange("(k p) n -> p k n", p=128))
            zl = singles.tile([128, 1, 2], BF, name="zl")
            yl = singles.tile([128, 1, 2], BF, name="yl")
            for h in range(2):
                yzl = psum_y.tile([128, SUB], F32, name="yzl",
                                  tag=f"y{h}")[:, 0:1]
                yhl = psum_y.tile([128, SUB], F32, name="yhl",
                                  tag=f"y{h}")[:, 0:1]
                for k in range(2):
                    nc.tensor.matmul(yzl, wz_sb[k][:, h * 128:(h + 1) * 128],
                                     xl[:, k, :], start=(k == 0), stop=(k == 1))
                    nc.tensor.matmul(yhl, wh_sb[k][:, h * 128:(h + 1) * 128],
                                     xl[:, k, :], start=(k == 0), stop=(k == 1))
                nc.scalar.activation(zl[:, :, h], yzl, ActFn.Sigmoid,
                                     bias=bz_sb[h][:, 0:1], scale=1.0)
                nc.scalar.activation(yl[:, :, h], yhl, ActFn.Identity,
                                     bias=bh_sb[h][:, 0:1], scale=1.0)
            al = singles.tile([128, 1, 2], BF, name="al")
            bl = singles.tile([128, 1, 2], BF, name="bl")
            nc.vector.tensor_scalar(al, zl, -1.0, 1.0,
                                    op0=AluOp.mult, op1=AluOp.add)
            nc.vector.tensor_tensor(bl, yl, zl, op=AluOp.mult)
            dl = singles.tile([128, 1, 2], BF, name="dl")
            sl = singles.tile([128, 1, 2], BF, name="sl")
            nc.vector.tensor_tensor(dl, otb[:, 3:4, :], al, op=AluOp.mult)
            nc.vector.tensor_tensor(sl, dl, bl, op=AluOp.add)
            pol = psum_o.tile([128, 2, SUB], F32, name="pol",
                              tag="po")[:, :, 0:1]
            for k in range(2):
                for oh in range(2):
                    nc.tensor.matmul(pol[:, oh, :],
                                     wo_sb[k][:, oh * 128:(oh + 1) * 128],
                                     sl[:, :, k], start=(k == 0), stop=(k == 1))
            osl = singles.tile([128, 2, 1], BF, name="osl")
            nc.scalar.copy(osl, pol)
            nc.sync.dma_start(
                out=out[:, T - 1:T].rearrange("(two p) n -> p two n", p=128),
                in_=osl)

        # ---- software-pipelined emission ----
        emit_phase1(0, 0, NSUB // 2)
        Aup_0 = lvl_pool.tile([128, LVL_TOTAL, 2], BF, name="Aup", tag="Au")
        Bup_0 = lvl_pool.tile([128, LVL_TOTAL, 2], BF, name="Bup", tag="Bu")
        emit_up_levels(0, Aup_0, Bup_0, range(NSPLIT), part=0)
        emit_phase1(0, NSUB // 2, NSUB)
        pend = None  # chunk c's out stage is emitted one iteration later:
        # its matmuls/evacs are then data-ready when PE/ACT dequeue them, so
        # the in-order PE/ACT queues never head-block the next chunk's
        # phase-1 stream behind an out burst that waits on the down-sweep.
        for c in range(NCHUNK):
            if c == 0:
                Aup_c, Bup_c = emit_up(0, Aup_0, Bup_0, tail_only=True)
            else:
                Aup_c, Bup_c = emit_up(c)
            emit_spine(c, Bup_c)
            if c + 1 < NCHUNK:
                emit_phase1(c + 1)
            else:
                emit_final()
            if pend is not None:
                emit_out(pend[0], pend[1], pend[2], 0, NSUB)
            dbuf_f, dbuf_r, emit_f = emit_down(c, Aup_c, Bup_c)
            emit_f(0, L // 4)
            emit_f(L // 4, L // 2)
            pend = (c, dbuf_f, dbuf_r)
        emit_out(pend[0], pend[1], pend[2], 0, NSUB // 2)
        emit_out(pend[0], pend[1], pend[2], NSUB // 2, NSUB)

    nc.compile()
    return nc


_NC_CACHE = {}


def _get_nc():
    if "nc" not in _NC_CACHE:
        _NC_CACHE["nc"] = build_nc()
    return _NC_CACHE["nc"]


def _prepare_in_maps(xs, Wz, bz, Wh, bh, Wo, bo):
    xs = np.asarray(xs, np.float32)
    Wz = np.asarray(Wz, np.float32)
    bz = np.asarray(bz, np.float32)
    Wh = np.asarray(Wh, np.float32)
    bh = np.asarray(bh, np.float32)
    Wo = np.asarray(Wo, np.float32)

    wall = np.concatenate([Wz.T, Wh.T, Wo.T], axis=1)
    wall = np.ascontiguousarray(wall).astype(BF16)
    ball = np.ascontiguousarray(
        np.stack([bz, -bz, bh], axis=1).astype(np.float32))

    in_maps = []
    for i in range(B):
        xti = np.ascontiguousarray(xs[i].T).astype(BF16)
        in_maps.append({"xt": xti, "wall": wall, "ball": ball})
    return in_maps


def _unblock(arr):
    """De-interleave the blocked [O, T] device output into true column
    order.  Block (c, s) occupies dram cols [base, base+512) with base =
    c*4096 - 1 + 512*s; within a block, dram col base+u holds true col
    base+2u (u<256, from even scan positions) or base+2(u-256)+1 (u>=256).
    c0s0 is shifted: dram cols 0..510 hold u=1..511.  Col T-1 is direct."""
    full = np.empty_like(arr)
    full[:, T - 1] = arr[:, T - 1]
    for c in range(NCHUNK):
        for s in range(NSUB):
            base = c * L - 1 + s * SUB
            if base < 0:
                blk = arr[:, 0:511]  # u = 1..511
                full[:, 1:511:2] = blk[:, 0:255]     # u=1..255 -> col 2u-1
                full[:, 0:512:2] = blk[:, 255:511]   # u=256..511 -> col 2(u-256)
            else:
                blk = arr[:, base:base + SUB]
                full[:, base:base + SUB:2] = blk[:, 0:256]
                full[:, base + 1:base + SUB:2] = blk[:, 256:512]
    return full


def _assemble(res, bo):
    bo = np.asarray(bo, np.float32)
    outs = []
    for i in range(B):
        arr = np.asarray(res.results[i]["out"]).astype(np.float32)
        outs.append(_unblock(arr).T + bo)
    return np.stack(outs, axis=0)


def run_traced(xs, Wz, bz, Wh, bh, Wo, bo, trace=True):
    in_maps = _prepare_in_maps(xs, Wz, bz, Wh, bh, Wo, bo)
    res = run_bass_kernel_spmd(_get_nc(), in_maps, core_ids=list(range(B)),
                               trace=trace)
    return _assemble(res, bo), res


def kernel(xs, Wz, bz, Wh, bh, Wo, bo):
    in_maps = _prepare_in_maps(xs, Wz, bz, Wh, bh, Wo, bo)
    res = run_bass_kernel_spmd(_get_nc(), in_maps, core_ids=list(range(B)))
    return _assemble(res, bo)


# revision 40
# speedup vs baseline: 1.0610x; 1.0610x over previous
"""MinGRU layer Trainium2 kernel (v3: dense down-sweep, blocked ev/od output).

Reference semantics (B=8, T=16384, D=H=O=256):
    zs = sigmoid(xs @ Wz.T + bz);  hs = xs @ Wh.T + bh
    a = concat([1], 1-zs);  b = concat([0], zs*hs)         (T+1 positions)
    states = jax.lax.associative_scan(combine, (a, b))[1][:, 1:]
    out = states @ Wo.T + bo
with combine((a0,b0),(a1,b1)) = (a0*b0, b0*a1 + b1) — NOT associative; the
result is defined by jax's odd/even recursion tree, replicated exactly
(modulo dropping |A|-products at tree level >= ADROP=5; bf16-sim rel err
0.0088, hardware 0.0081, vs the 2e-2 gate).

v3 layout/scheduling notes (vs v2):
  - Packed halves [128, pos, 2] bf16 everywhere (2x_1p DVE mode).  Probe
    measurements: stride-2-position access keeps full DVE speed (TT 2x,
    TS/copy 4x); stride >= 4-position drops to 1x.  The v2 down-sweep wrote
    dbuf at strides 4..4096 (1x); v3 keeps every access dense or stride-2.
  - Down-sweep: dense cascade R_6..R_1 where R_l[i] = scan value at chunk
    position i*2^l - 1 (R_l[0] = chunk prefix).  R_6 = [prefix, B_6];
    level l: odds of R_l by FMA from R_{l+1} (stride-2 writes, 2x), evens
    by copy of R_{l+1} (4x).  Final level-0 FMA produces dbuf_f (dense).
  - Output is stored BLOCKED per 512-col sub: first 256 psum cols = even
    scan positions (from dbuf_f), next 256 = odd (from dbuf_r = R_1); the
    host de-interleaves (free) and adds bo.
  - a = 1-z on DVE tensor_scalar (4x) instead of ACT sigmoid(-x) (frees ACT).
  - b = z*h: pipeline-head subs (chunk 0 s<3) via DVE STT straight from
    PSUM (short critical chain); everywhere else ACT evacuates yh+bh and
    DVE does a 2x TT (2.3x cheaper per sub on DVE).
  - psum_y/psum_o both double-buffered (4+4 of the 8 PSUM banks) so the
    out-matmuls of sub s+1 overlap sub s's evacuation.
  - Chunk c's out stage (matmuls+evac+DMA) is emitted one loop iteration
    LATE, after phase1(c+1): its ops are then data-ready when the in-order
    PE/ACT queues reach them, so an out burst waiting on the down-sweep
    never head-blocks the next chunk's phase-1 stream.
  - Head DMAs issue from the ACT/GPSIMD queues (parallel issue, the SP
    queue serializes at ~640ns per DMA).
Measured (8 cores, NTFF): ~178-182us vs 190.2us for v2, rel err 0.00813
(ADROP=5; ADROP=6 gives 0.00570 at +2.5us).  Beware: pool depth changes
(zy 4->5, x 4->6) regress 10-30us via SBUF layout shifts, and the machine
occasionally has ~186-215us slow episodes unrelated to code.
Sharding: batch b=8 across the 8 cores (one sequence per core); weights
replicated.  The host pre-transposes/casts x and the weights.
"""

from contextlib import ExitStack

import numpy as np
import ml_dtypes

import concourse.bacc as bacc
import concourse.tile as tile
from concourse import mybir
from concourse.bass_utils import run_bass_kernel_spmd

BF16 = ml_dtypes.bfloat16
F32 = mybir.dt.float32
BF = mybir.dt.bfloat16

B, T, D, H, O = 8, 16384, 256, 256, 256
L = 4096          # positions per chunk (power of 2)
NCHUNK = T // L   # 4 full chunks; position T (=16384) handled separately
SUB = 512         # matmul sub-chunk (one PSUM bank at f32)
NSUB = L // SUB   # 8
ADROP = 5         # tree levels >= ADROP: |A| products are dropped
                  # (bf16-sim rel err 0.0088 vs 0.0041 at ADROP=6; gate 2e-2)

AluOp = mybir.AluOpType
ActFn = mybir.ActivationFunctionType

# level arrays for lvl 1..ADROP (Aup/Bup): sizes L/2..L/64
LVL_OFF = {}
_o = 0
for _l in range(1, ADROP + 1):
    LVL_OFF[_l] = _o
    _o += L >> _l
LVL_TOTAL = _o  # 4032


def build_nc():
    nc = bacc.Bacc()

    xt = nc.dram_tensor("xt", [D, T], BF, kind="ExternalInput")
    wall = nc.dram_tensor("wall", [D, 3 * H], BF, kind="ExternalInput")
    ball = nc.dram_tensor("ball", [H, 3], F32, kind="ExternalInput")
    out = nc.dram_tensor("out", [O, T], BF, kind="ExternalOutput")

    with tile.TileContext(nc) as tc, ExitStack() as ctx:
        singles = ctx.enter_context(tc.tile_pool(name="singles", bufs=1))
        ab_pool = ctx.enter_context(tc.tile_pool(name="ab", bufs=2))
        lvl_pool = ctx.enter_context(tc.tile_pool(name="lvl", bufs=1))
        dbuf_pool = ctx.enter_context(tc.tile_pool(name="dbuf", bufs=2))
        rsm_pool = ctx.enter_context(tc.tile_pool(name="rsm", bufs=2))
        tmp_pool = ctx.enter_context(tc.tile_pool(name="tmp", bufs=2))
        zy_pool = ctx.enter_context(tc.tile_pool(name="zy", bufs=4))
        x_pool = ctx.enter_context(tc.tile_pool(name="xp", bufs=4))
        osb_pool = ctx.enter_context(tc.tile_pool(name="osb", bufs=3))
        psum_y = ctx.enter_context(tc.tile_pool(name="psy", bufs=2, space="PSUM"))
        psum_o = ctx.enter_context(tc.tile_pool(name="pso", bufs=2, space="PSUM"))

        # ---- prefetch the first x sub-tile before the weight DMAs so its
        # transfer overlaps them (shortens the head-of-kernel latency) ----
        # head DMAs issue from different engine queues so they don't
        # serialize behind each other on the SP queue (~640ns issue each)
        xk_pre = []
        for s0, eng in ((0, nc.gpsimd), (1, nc.scalar)):
            xp = x_pool.tile([128, 2, SUB], BF, name="xk", tag="xk")
            eng.dma_start(
                out=xp[:, :, :SUB],
                in_=xt[:, s0 * SUB:(s0 + 1) * SUB].rearrange(
                    "(k p) n -> p k n", p=128))
            xk_pre.append(xp)

        # ---- constants: one DMA for all weights, one for all biases ----
        wsb = singles.tile([128, 2, 3 * H], BF, name="wsb")
        nc.scalar.dma_start(out=wsb,
                            in_=wall[:, :].rearrange("(k p) n -> p k n", p=128))
        wz_sb = [wsb[:, k, 0:H] for k in range(2)]
        wh_sb = [wsb[:, k, H:2 * H] for k in range(2)]
        wo_sb = [wsb[:, k, 2 * H:3 * H] for k in range(2)]
        bsb = singles.tile([128, 2, 3], F32, name="bsb")
        nc.sync.dma_start(out=bsb,
                          in_=ball[:, :].rearrange("(h p) n -> p h n", p=128))
        bz_sb = [bsb[:, h, 0:1] for h in range(2)]
        bzn_sb = [bsb[:, h, 1:2] for h in range(2)]
        bh_sb = [bsb[:, h, 2:3] for h in range(2)]

        # cross-chunk bookkeeping, halves packed innermost: [128, n, 2]
        otb = singles.tile([128, 4, 2], BF, name="otb")

        abufs = {}

        def emit_phase1(c, s_lo=0, s_hi=NSUB):
            """DMA x, matmuls, sigmoid z, a = 1-z (DVE TS), b = z*h for
            chunk c, subs [s_lo, s_hi).  Chunk 0 runs b on DVE (STT from
            PSUM, per sub) to shorten the head critical chain; chunks >=1
            use ACT yh-evac + DVE TT except the last sub (STT)."""
            if s_lo == 0:
                a_buf = ab_pool.tile([128, L, 2], BF, name="a_buf", tag="a")
                b_buf = ab_pool.tile([128, L, 2], BF, name="b_buf", tag="b")
                abufs[c] = (a_buf, b_buf)
                if c == 0:
                    nc.vector.memset(a_buf[:, 0:1, :], 1.0)
                    nc.vector.memset(b_buf[:, 0:1, :], 0.0)
            else:
                a_buf, b_buf = abufs[c]

            for s in range(s_lo, s_hi):
                if c == 0:
                    x0 = s * SUB
                    ncols = SUB if s < NSUB - 1 else SUB - 1
                    acol = s * SUB + 1
                else:
                    x0 = c * L - 1 + s * SUB
                    ncols = SUB
                    acol = s * SUB
                if c == 0 and s < 2:
                    xk = xk_pre[s]
                else:
                    xk = x_pool.tile([128, 2, SUB], BF, name="xk", tag="xk")
                    nc.sync.dma_start(
                        out=xk[:, :, :ncols],
                        in_=xt[:, x0:x0 + ncols].rearrange("(k p) n -> p k n",
                                                           p=128))
                # A,C,B,D,A,C,B,D over 4 banks: every same-bank revisit is
                # 4 apart, so no matmul pays the PSUM-bank half-rate bubble
                yzp = [psum_y.tile([128, SUB], F32, name=f"yz{h}", tag=f"y{h}")
                       for h in range(2)]
                yhp = [psum_y.tile([128, SUB], F32, name=f"yh{h}", tag=f"y{h}")
                       for h in range(2)]
                for k in range(2):
                    st, sp = k == 0, k == 1
                    for h in range(2):
                        nc.tensor.matmul(yzp[h][:, :ncols],
                                         wz_sb[k][:, h * 128:(h + 1) * 128],
                                         xk[:, k, :ncols], start=st, stop=sp)
                    for h in range(2):
                        nc.tensor.matmul(yhp[h][:, :ncols],
                                         wh_sb[k][:, h * 128:(h + 1) * 128],
                                         xk[:, k, :ncols], start=st, stop=sp)
                zt = zy_pool.tile([128, SUB, 2], BF, name="zt", tag="zt")
                # STT (1x from PSUM, no ACT dependency) only where the
                # pipeline head latency matters; TT elsewhere is 2.3x
                # cheaper on DVE
                use_stt = c == 0 and s < 3
                yhs = None
                if not use_stt:
                    yhs = zy_pool.tile([128, SUB, 2], BF, name="yhs",
                                       tag="yhs")
                for h in range(2):
                    nc.scalar.activation(zt[:, :ncols, h], yzp[h][:, :ncols],
                                         ActFn.Sigmoid, bias=bz_sb[h][:, 0:1],
                                         scale=1.0)
                    if use_stt:
                        # b = (yh+bh)*z on DVE straight from PSUM (1x)
                        nc.vector.scalar_tensor_tensor(
                            b_buf[:, acol:acol + ncols, h],
                            yhp[h][:, :ncols], bh_sb[h][:, 0:1],
                            zt[:, :ncols, h],
                            op0=AluOp.add, op1=AluOp.mult)
                    else:
                        nc.scalar.activation(yhs[:, :ncols, h],
                                             yhp[h][:, :ncols], ActFn.Identity,
                                             bias=bh_sb[h][:, 0:1], scale=1.0)
                # a = 1 - z on DVE tensor_scalar (4x mode)
                nc.vector.tensor_scalar(a_buf[:, acol:acol + ncols, :],
                                        zt[:, :ncols, :], -1.0, 1.0,
                                        op0=AluOp.mult, op1=AluOp.add)
                if not use_stt:
                    nc.vector.tensor_tensor(b_buf[:, acol:acol + ncols, :],
                                            yhs[:, :ncols, :],
                                            zt[:, :ncols, :], op=AluOp.mult)

        NSPLIT = 5  # chunk-0 up levels emitted per half (head pipelining)

        def emit_up_levels(c, Aup, Bup, levels, part=None):
            a_buf, b_buf = abufs[c]
            for lvl in levels:
                n = L >> lvl
                m = n // 2
                if lvl == 0:
                    sA, sB = a_buf, b_buf
                else:
                    o = LVL_OFF[lvl]
                    sA = Aup[:, o:o + n, :]
                    sB = Bup[:, o:o + n, :]
                o2 = LVL_OFF[lvl + 1]
                if part is None:
                    lo, hi = 0, m
                else:
                    lo, hi = (0, m // 2) if part == 0 else (m // 2, m)
                dB = Bup[:, o2 + lo:o2 + hi, :]
                A_ev = sA[:, 2 * lo:2 * hi:2, :]
                A_od = sA[:, 2 * lo + 1:2 * hi:2, :]
                B_ev = sB[:, 2 * lo:2 * hi:2, :]
                B_od = sB[:, 2 * lo + 1:2 * hi:2, :]
                if lvl + 1 <= ADROP - 1:
                    # A of the next level is consumed (up B-chain + down FMA)
                    dA = Aup[:, o2 + lo:o2 + hi, :]
                    nc.vector.tensor_tensor(dA, A_ev, B_ev, op=AluOp.mult)
                tu = tmp_pool.tile([128, L // 2, 2], BF, name="tu", tag="tmp")
                nc.vector.tensor_tensor(tu[:, lo:hi, :], B_ev, A_od,
                                        op=AluOp.mult)
                nc.vector.tensor_tensor(dB, tu[:, lo:hi, :], B_od,
                                        op=AluOp.add)

        def emit_up(c, Aup=None, Bup=None, tail_only=False):
            if Aup is None:
                Aup = lvl_pool.tile([128, LVL_TOTAL, 2], BF, name="Aup",
                                    tag="Au")
                Bup = lvl_pool.tile([128, LVL_TOTAL, 2], BF, name="Bup",
                                    tag="Bu")
            if tail_only:
                emit_up_levels(c, Aup, Bup, range(NSPLIT), part=1)
                emit_up_levels(c, Aup, Bup, range(NSPLIT, ADROP))
            else:
                emit_up_levels(c, Aup, Bup, range(ADROP))
            return Aup, Bup

        def emit_spine(c, Bup):
            # |A_12| ~ |ab|*prod(B)^11 is far below bf16 resolution, so the
            # cross-chunk combines collapse: otb[c] = B_12 = B_6[last]
            o6 = LVL_OFF[ADROP]
            n6 = L >> ADROP
            nc.vector.tensor_copy(otb[:, c:c + 1, :],
                                  Bup[:, o6 + n6 - 1:o6 + n6, :])

        def emit_down(c, Aup, Bup):
            """Dense cascade: R_l[i] = scan at chunk position i*2^l - 1
            (R_l[0] = prefix).  Returns (dbuf_f, dbuf_r, emit_f):
            dbuf_f[i] = scan[2i]  (chunk-local, i=0..L/2-1)
            dbuf_r[i] = scan[2i-1] (i=0..L/2; [0]=prefix, [L/2]=chunk top).
            All writes dense or stride-2 (full DVE rate).  The final level-0
            FMA into dbuf_f is emitted by calling emit_f(lo, hi) so the
            tail can interleave FMA slices with out-matmul subs."""
            a_buf, b_buf = abufs[c]
            n6 = L >> ADROP
            o6 = LVL_OFF[ADROP]
            r_prev = rsm_pool.tile([128, n6 + 1, 2], BF, name="r6", tag="r6")
            if c == 0:
                nc.vector.memset(r_prev[:, 0:1, :], 0.0)
            else:
                nc.vector.tensor_copy(r_prev[:, 0:1, :], otb[:, c - 1:c, :])
            nc.vector.tensor_copy(r_prev[:, 1:n6 + 1, :],
                                  Bup[:, o6:o6 + n6, :])
            for lvl in range(ADROP - 1, 0, -1):
                n = L >> lvl          # positions at this level
                m = n // 2
                o = LVL_OFF[lvl]
                A_ev = Aup[:, o:o + n:2, :] if lvl > 0 else None
                B_ev = Bup[:, o:o + n:2, :]
                rl = rsm_pool.tile([128, n + 1, 2], BF, name=f"r{lvl}",
                                   tag=f"r{lvl}") if lvl > 1 else \
                    dbuf_pool.tile([128, L // 2 + 1, 2], BF, name="dbuf_r",
                                   tag="dr")
                # odds: R_l[2i+1] = R_{l+1}[i] * A_l[2i] + B_l[2i]
                nc.vector.tensor_tensor(rl[:, 1:n + 1:2, :],
                                        r_prev[:, 0:m, :], A_ev,
                                        op=AluOp.mult)
                nc.vector.tensor_tensor(rl[:, 1:n + 1:2, :],
                                        rl[:, 1:n + 1:2, :], B_ev,
                                        op=AluOp.add)
                # evens: R_l[2i] = R_{l+1}[i]
                nc.vector.tensor_copy(rl[:, 0:n + 1:2, :], r_prev)
                r_prev = rl
            dbuf_r = r_prev
            # level 0: dbuf_f[i] = R_1[i] * a_buf[2i] + b_buf[2i] (the gate
            # arrays are filled with the chunk offset baked in, so the even
            # slots are correct for every chunk; chunk 0's scan[0]=0 falls
            # out of the a_buf[0]=1 / b_buf[0]=0 memsets).
            dbuf_f = dbuf_pool.tile([128, L // 2, 2], BF, name="dbuf_f",
                                    tag="df")

            def emit_f(lo, hi):
                ga = a_buf[:, 2 * lo:2 * hi:2, :]
                gb = b_buf[:, 2 * lo:2 * hi:2, :]
                nc.vector.tensor_tensor(dbuf_f[:, lo:hi, :],
                                        dbuf_r[:, lo:hi, :], ga,
                                        op=AluOp.mult)
                nc.vector.tensor_tensor(dbuf_f[:, lo:hi, :],
                                        dbuf_f[:, lo:hi, :], gb,
                                        op=AluOp.add)

            return dbuf_f, dbuf_r, emit_f

        def emit_out(c, dbuf_f, dbuf_r, s_lo=0, s_hi=NSUB):
            """Out-matmuls per 512-col sub in BLOCKED order: psum cols
            0..255 = even scan positions (dbuf_f), 256..511 = odd (dbuf_r).
            DRAM block for (c, s) starts at col c*L - 1 + 512*s (+1 skip
            for c0s0)."""
            last = c == NCHUNK - 1
            Q = SUB // 2  # 256
            for s in range(s_lo, s_hi):
                f_sl = dbuf_f[:, Q * s:Q * s + Q, :]
                r_sl = dbuf_r[:, Q * s + 1:Q * s + 1 + Q, :]
                if last and s % 2 == 1:
                    # tail: phase1 is over, reuse the idle psum_y banks so
                    # consecutive subs' matmuls/evacs fully overlap
                    p0 = psum_y.tile([128, SUB], F32, name="poa0", tag="y0")
                    p1 = psum_y.tile([128, SUB], F32, name="poa1", tag="y1")
                    po_h = [p0, p1]
                else:
                    po = psum_o.tile([128, 2, SUB], F32, name="po", tag="po")
                    po_h = [po[:, 0, :], po[:, 1, :]]
                # complete each psum column-region's k0->k1 accumulation
                # before starting the other region of the same bank (an
                # interleaved second `start` clobbers the pending group)
                for sl, q0 in ((f_sl, 0), (r_sl, Q)):
                    for k in range(2):
                        st, sp = k == 0, k == 1
                        for oh in range(2):
                            nc.tensor.matmul(
                                po_h[oh][:, q0:q0 + Q],
                                wo_sb[k][:, oh * 128:(oh + 1) * 128],
                                sl[:, :, k], start=st, stop=sp)
                base = c * L - 1 + s * SUB
                skip = 1 if (c == 0 and s == 0) else 0
                dst = out[:, base + skip:base + SUB]
                if last:
                    # tail: split every sub's evacuation across ACT and the
                    # now-idle DVE into one tile (single DMA per sub), and
                    # alternate the DMA issue queue (SP serializes issues at
                    # ~640ns each)
                    o2 = osb_pool.tile([128, 2, SUB], BF, name="osb2",
                                       tag="osb")
                    nc.scalar.copy(o2[:, 0, :], po_h[0])
                    nc.vector.tensor_copy(o2[:, 1, :], po_h[1])
                    nc.sync.dma_start(
                        out=dst.rearrange("(two p) n -> p two n", p=128),
                        in_=o2[:, :, skip:])
                else:
                    osb = osb_pool.tile([128, 2, SUB], BF, name="osb",
                                        tag="osb")
                    nc.scalar.copy(osb, po)
                    nc.sync.dma_start(
                        out=dst.rearrange("(two p) n -> p two n", p=128),
                        in_=osb[:, :, skip:])

        def emit_final():
            # final position T: scan[T] = otb[3]*a_T + b_T fed to Wo;
            # out col T-1 written directly (host treats it as a singleton).
            xl = singles.tile([128, 2, 1], BF, name="xl")
            nc.sync.dma_start(
                out=xl,
                in_=xt[:, T - 1:T].rearrange("(k p) n -> p k n", p=128))
            zl = singles.tile([128, 1, 2], BF, name="zl")
            yl = singles.tile([128, 1, 2], BF, name="yl")
            for h in range(2):
                yzl = psum_y.tile([128, SUB], F32, name="yzl",
                                  tag=f"y{h}")[:, 0:1]
                yhl = psum_y.tile([128, SUB], F32, name="yhl",
                                  tag=f"y{h}")[:, 0:1]
                for k in range(2):
                    nc.tensor.matmul(yzl, wz_sb[k][:, h * 128:(h + 1) * 128],
                                     xl[:, k, :], start=(k == 0), stop=(k == 1))
                    nc.tensor.matmul(yhl, wh_sb[k][:, h * 128:(h + 1) * 128],
                                     xl[:, k, :], start=(k == 0), stop=(k == 1))
                nc.scalar.activation(zl[:, :, h], yzl, ActFn.Sigmoid,
                                     bias=bz_sb[h][:, 0:1], scale=1.0)
                nc.scalar.activation(yl[:, :, h], yhl, ActFn.Identity,
                                     bias=bh_sb[h][:, 0:1], scale=1.0)
            al = singles.tile([128, 1, 2], BF, name="al")
            bl = singles.tile([128, 1, 2], BF, name="bl")
            nc.vector.tensor_scalar(al, zl, -1.0, 1.0,
                                    op0=AluOp.mult, op1=AluOp.add)
            nc.vector.tensor_tensor(bl, yl, zl, op=AluOp.mult)
            dl = singles.tile([128, 1, 2], BF, name="dl")
            sl = singles.tile([128, 1, 2], BF, name="sl")
            nc.vector.tensor_tensor(dl, otb[:, 3:4, :], al, op=AluOp.mult)
            nc.vector.tensor_tensor(sl, dl, bl, op=AluOp.add)
            pol = psum_o.tile([128, 2, SUB], F32, name="pol",
                              tag="po")[:, :, 0:1]
            for k in range(2):
                for oh in range(2):
                    nc.tensor.matmul(pol[:, oh, :],
                                     wo_sb[k][:, oh * 128:(oh + 1) * 128],
                                     sl[:, :, k], start=(k == 0), stop=(k == 1))
            osl = singles.tile([128, 2, 1], BF, name="osl")
            nc.scalar.copy(osl, pol)
            nc.sync.dma_start(
                out=out[:, T - 1:T].rearrange("(two p) n -> p two n", p=128),
                in_=osl)

        # ---- software-pipelined emission ----
        emit_phase1(0, 0, NSUB // 2)
        Aup_0 = lvl_pool.tile([128, LVL_TOTAL, 2], BF, name="Aup", tag="Au")
        Bup_0 = lvl_pool.tile([128, LVL_TOTAL, 2], BF, name="Bup", tag="Bu")
        emit_up_levels(0, Aup_0, Bup_0, range(NSPLIT), part=0)
        emit_phase1(0, NSUB // 2, NSUB)
        pend = None  # chunk c's out stage is emitted one iteration later:
        # its matmuls/evacs are then data-ready when PE/ACT dequeue them, so
        # the in-order PE/ACT queues never head-block the next chunk's
        # phase-1 stream behind an out burst that waits on the down-sweep.
        for c in range(NCHUNK):
            if c == 0:
                Aup_c, Bup_c = emit_up(0, Aup_0, Bup_0, tail_only=True)
            else:
                Aup_c, Bup_c = emit_up(c)
            emit_spine(c, Bup_c)
            if c + 1 < NCHUNK:
                emit_phase1(c + 1)
            else:
                emit_final()
            if pend is not None:
                emit_out(pend[0], pend[1], pend[2], 0, NSUB)
            dbuf_f, dbuf_r, emit_f = emit_down(c, Aup_c, Bup_c)
            emit_f(0, L // 4)
            emit_f(L // 4, L // 2)
            pend = (c, dbuf_f, dbuf_r)
        emit_out(pend[0], pend[1], pend[2], 0, NSUB // 2)
        emit_out(pend[0], pend[1], pend[2], NSUB // 2, NSUB)

    nc.compile()
    return nc


_NC_CACHE = {}


def _get_nc():
    if "nc" not in _NC_CACHE:
        _NC_CACHE["nc"] = build_nc()
    return _NC_CACHE["nc"]


def _prepare_in_maps(xs, Wz, bz, Wh, bh, Wo, bo):
    xs = np.asarray(xs, np.float32)
    Wz = np.asarray(Wz, np.float32)
    bz = np.asarray(bz, np.float32)
    Wh = np.asarray(Wh, np.float32)
    bh = np.asarray(bh, np.float32)
    Wo = np.asarray(Wo, np.float32)

    wall = np.concatenate([Wz.T, Wh.T, Wo.T], axis=1)
    wall = np.ascontiguousarray(wall).astype(BF16)
    ball = np.ascontiguousarray(
        np.stack([bz, -bz, bh], axis=1).astype(np.float32))

    in_maps = []
    for i in range(B):
        xti = np.ascontiguousarray(xs[i].T).astype(BF16)
        in_maps.append({"xt": xti, "wall": wall, "ball": ball})
    return in_maps


def _unblock(arr):
    """De-interleave the blocked [O, T] device output into true column
    order.  Block (c, s) occupies dram cols [base, base+512) with base =
    c*4096 - 1 + 512*s; within a block, dram col base+u holds true col
    base+2u (u<256, from even scan positions) or base+2(u-256)+1 (u>=256).
    c0s0 is shifted: dram cols 0..510 hold u=1..511.  Col T-1 is direct."""
    full = np.empty_like(arr)
    full[:, T - 1] = arr[:, T - 1]
    for c in range(NCHUNK):
        for s in range(NSUB):
            base = c * L - 1 + s * SUB
            if base < 0:
                blk = arr[:, 0:511]  # u = 1..511
                full[:, 1:511:2] = blk[:, 0:255]     # u=1..255 -> col 2u-1
                full[:, 0:512:2] = blk[:, 255:511]   # u=256..511 -> col 2(u-256)
            else:
                blk = arr[:, base:base + SUB]
                full[:, base:base + SUB:2] = blk[:, 0:256]
                full[:, base + 1:base + SUB:2] = blk[:, 256:512]
    return full


def _assemble(res, bo):
    bo = np.asarray(bo, np.float32)
    outs = []
    for i in range(B):
        arr = np.asarray(res.results[i]["out"]).astype(np.float32)
        outs.append(_unblock(arr).T + bo)
    return np.stack(outs, axis=0)


def run_traced(xs, Wz, bz, Wh, bh, Wo, bo, trace=True):
    in_maps = _prepare_in_maps(xs, Wz, bz, Wh, bh, Wo, bo)
    res = run_bass_kernel_spmd(_get_nc(), in_maps, core_ids=list(range(B)),
                               trace=trace)
    return _assemble(res, bo), res


def kernel(xs, Wz, bz, Wh, bh, Wo, bo):
    in_maps = _prepare_in_maps(xs, Wz, bz, Wh, bh, Wo, bo)
    res = run_bass_kernel_spmd(_get_nc(), in_maps, core_ids=list(range(B)))
    return _assemble(res, bo)
